# revision 20
# baseline (speedup 1.0000x reference)
"""DynamicConv1D Trainium2 kernel.

Reference computation (per batch b):
  dw = conv1d(x, W, pad=3) + b            # [O*I*K, T] dynamic weights
  dw = softmax(dw.reshape(O,I,K,T)/sqrt(K), axis=K)
  y[o,t] = sum_{i,k} x[i, t+k-3] * dw[o,i,k,t]

Sharding: 8 cores = 4 batches x 2 halves of O (16 out-channels each).
Each core gets x[b] plus its half of the (rearranged) conv weights and
computes y[b, half*16:(half+1)*16, :]. No collectives; the host scatters
inputs and concatenates outputs.

Per-core layout (t-tile = 128 positions on partitions):
  conv as matmul: dw[t, (k,o,i)] = sum_{(j,c)} X1[(j,c), t] * W'[(j,c), (k,o,i)]
    X1[(j,c), u] = x[c, u+j-3]  (im2col built host-side, bf16); ones row in
    X1b so the bias rides as an extra W' row; 1/sqrt(K) folded into W'/b.
  psum is organized as 4 bank groups (2+2+2+1 banks) so ScalarE drains dw
  with four wide exp's per tile.  x_unf ships from the host as a
  [tp, tile, (k,i)] tensor (no on-device DMA transposes).
  eex is PAIR-sized [t, pair, k, {e,ex}, (o,i)] so every k-sum tree level
  runs as one wide tensor_tensor add per tile-pair (DVE 2x mode: all
  operands bf16, packed; scalar_tensor_tensor / f32 operands would drop
  to 1x on hardware).
  Tail per pair stays bf16: r = 1/den via exponent-flip seed (int16
  tensor_scalar, 4x) + one Newton step (2x muls), y1 = num*r (2x),
  y[t,o] = reduce_i y1 after one 2x i-halving.
"""

import numpy as np

B = 4
C = 32
K = 7
T = 4096
O_FULL = 32
OH = 16  # out-channels per core
PAD = 3
TT = 128  # t positions per tile (partition dim)
FREE = K * OH * C  # 3584, matmul free index = k*512 + o*32 + i
SLAB = OH * C  # 512, one k-slab
CD1 = 128  # (j, c) rows for j=0..3
CD2 = 97  # (j, c) rows for j=4..6 plus ones row
CHUNK = 512  # psum chunk (1 bank); FREE = 7*CHUNK
KI = K * C  # 224
RECIP_MAGIC = 0x7EF4  # bf16 exponent-flip reciprocal seed (top16 of 0x7EF477D5)

_prog_cache = {}


def _build(t_len):
    """Build and compile the per-core Bass program for sequence length t_len."""
    import concourse.tile as tile
    from concourse import bacc, mybir

    nt = t_len // TT
    nc = bacc.Bacc("TRN2", target_bir_lowering=False, debug=False, num_devices=1)
    f32 = mybir.dt.float32
    bf16 = mybir.dt.bfloat16
    i16 = mybir.dt.int16
    mult = mybir.AluOpType.mult
    add = mybir.AluOpType.add

    padw = t_len + 8
    xp_d = nc.dram_tensor("xpad", [C + 1, padw], bf16, kind="ExternalInput").ap()
    xt_d = nc.dram_tensor("xt", [TT, nt * C], bf16, kind="ExternalInput").ap()
    w1_d = nc.dram_tensor("wp1", [CD1, FREE], bf16, kind="ExternalInput").ap()
    w2_d = nc.dram_tensor("wp2", [CD2, FREE], bf16, kind="ExternalInput").ap()
    y_d = nc.dram_tensor("yout", [TT, nt * OH], f32, kind="ExternalOutput").ap()

    with tile.TileContext(nc) as tc:
        with (
            tc.tile_pool(name="const", bufs=1) as cpool,
            tc.tile_pool(name="ep", bufs=2) as epool,
            tc.tile_pool(name="tree", bufs=2) as tpool,
            tc.tile_pool(name="small", bufs=2) as spool,
            tc.tile_pool(name="psum", bufs=1, space="PSUM") as ppool,
        ):
            xp = cpool.tile([C + 1, padw], bf16, tag="xp")
            xt = cpool.tile([TT, nt, C], bf16, tag="xt")
            x1a = cpool.tile([CD1, t_len], bf16, tag="x1a")
            x1b = cpool.tile([CD2, t_len], bf16, tag="x1b")
            w1 = cpool.tile([CD1, FREE], bf16, tag="w1")
            w2 = cpool.tile([CD2, FREE], bf16, tag="w2")
            x2h = cpool.tile([TT, K, nt, C], bf16, tag="x2h")
            y_sb = cpool.tile([TT, nt * OH], f32, tag="ysb")

            # Startup: only xp/xt/W come from (8-core contended) HBM; the
            # im2col tensors x1a/x1b and the unfold x2h are fanned out from
            # xp/xt with on-chip SBUF->SBUF DMAs, head (first tiles) first.
            # DMA triggers stay off the scalar queue (they steal act time).
            hf = 1792
            nt_ = nt
            HB = min(8, nt_)  # tile blocks in the head fanout
            HC = min(1024, t_len)  # head columns for x1 strips
            nc.sync.dma_start(xp[:], xp_d)
            nc.gpsimd.dma_start(xt[:], xt_d)

            # x1 head strips on the scalar queue: the trigger engine-time
            # lands before the first acts need ScalarE, and it frees sync
            # for the weight stream that gates tile 0.
            for j in range(K):
                tgt, r0 = (x1a, j * C) if j < 4 else (x1b, (j - 4) * C)
                nc.scalar.dma_start(
                    tgt[r0 : r0 + C, 0:HC], xp[0:C, j : j + HC]
                )
            nc.scalar.dma_start(x1b[CD2 - 1 : CD2, 0:HC], xp[C : C + 1, 3 : 3 + HC])
            # x2h head (tile blocks 0:HB): x2h[tp, tt, k*C:(k+1)*C] from xt
            # with a (k-3) partition shift; block-crossing rows come from the
            # neighbor block; out-of-range rows are zero padding.
            for k in range(K):
                dlt = k - PAD
                if dlt <= 0:
                    if dlt < 0:
                        nc.vector.memset(x2h[:, k, 0, :], 0.0)
                    nc.gpsimd.dma_start(
                        x2h[-dlt : TT, k, 0:HB, :], xt[0 : TT + dlt, 0:HB, :]
                    )
                    if dlt < 0:
                        nc.gpsimd.dma_start(
                            x2h[0:-dlt, k, 1:HB, :], xt[TT + dlt : TT, 0 : HB - 1, :]
                        )
                else:
                    nc.gpsimd.dma_start(
                        x2h[0 : TT - dlt, k, 0:HB, :], xt[dlt:TT, 0:HB, :]
                    )
                    nc.gpsimd.dma_start(
                        x2h[TT - dlt : TT, k, 0:HB, :], xt[0:dlt, 1 : HB + 1, :]
                    )
            # weights (tile 0's matmuls gate on these) — all on the fast
            # sync hw-DGE queue; the sw-DGE queue is ~2x slower for these
            # strided loads and stays dedicated to the x2h fanout.
            nc.sync.dma_start(w1[:, 0:hf], w1_d[:, 0:hf])
            nc.sync.dma_start(w2[:, 0:hf], w2_d[:, 0:hf])
            nc.sync.dma_start(w1[:, hf:], w1_d[:, hf:])
            nc.sync.dma_start(w2[:, hf:], w2_d[:, hf:])

            # tails
            if HC < t_len:
                for j in range(K):
                    tgt, r0 = (x1a, j * C) if j < 4 else (x1b, (j - 4) * C)
                    nc.sync.dma_start(
                        tgt[r0 : r0 + C, HC:t_len], xp[0:C, j + HC : j + t_len]
                    )
                nc.sync.dma_start(
                    x1b[CD2 - 1 : CD2, HC:t_len], xp[C : C + 1, 3 + HC : 3 + t_len]
                )
            for k in range(K if HB < nt else 0):
                dlt = k - PAD
                if dlt <= 0:
                    nc.gpsimd.dma_start(
                        x2h[-dlt : TT, k, HB:nt, :], xt[0 : TT + dlt, HB:nt, :]
                    )
                    if dlt < 0:
                        nc.gpsimd.dma_start(
                            x2h[0:-dlt, k, HB:nt, :],
                            xt[TT + dlt : TT, HB - 1 : nt - 1, :],
                        )
                else:
                    nc.vector.memset(x2h[:, k, nt - 1, :], 0.0)
                    nc.gpsimd.dma_start(
                        x2h[0 : TT - dlt, k, HB:nt, :], xt[dlt:TT, HB:nt, :]
                    )
                    if HB + 1 <= nt - 1:
                        nc.gpsimd.dma_start(
                            x2h[TT - dlt : TT, k, HB : nt - 1, :],
                            xt[0:dlt, HB + 1 : nt, :],
                        )

            for tt in range(nt):
                t0 = tt * TT
                sp = tt % 2  # pair slot
                x1at = x1a[:, t0 : t0 + TT]
                x1bt = x1b[:, t0 : t0 + TT]

                # psum: 3 double-bank groups + 1 single bank (7 chunks of 512)
                pg = [
                    ppool.tile([TT, 1024], f32, tag="pA", name="pA"),
                    ppool.tile([TT, 1024], f32, tag="pB", name="pB"),
                    ppool.tile([TT, 1024], f32, tag="pC", name="pC"),
                    ppool.tile([TT, 512], f32, tag="pD", name="pD"),
                ]

                def chunk_ap(ci):
                    g, o = divmod(ci, 2)
                    return pg[g][:, o * 512 : (o + 1) * 512]

                # Interleave the two contraction halves per chunk so each
                # chunk (and its act) completes as early as possible.
                for ci in range(K):
                    cs = slice(ci * CHUNK, (ci + 1) * CHUNK)
                    nc.tensor.matmul(
                        chunk_ap(ci), x1at, w1[:, cs], start=True, stop=False
                    )
                    nc.tensor.matmul(
                        chunk_ap(ci), x1bt, w2[:, cs], start=False, stop=True
                    )

                # eex[t, pair, k, {e, ex}, (o,i)]
                if sp == 0:
                    eex = epool.tile([TT, 2, K, 2, SLAB], bf16, tag="eex")
                for g in range(4):
                    kw = 2 if g < 3 else 1  # k-slabs in this group
                    src = pg[g][:].rearrange("p (k q) -> p k q", k=kw)
                    nc.scalar.activation(
                        eex[:, sp, 2 * g : 2 * g + kw, 0, :],
                        src,
                        mybir.ActivationFunctionType.Exp,
                    )

                # EX = e * x_unf broadcast over o (one wide 2x TT op per tile)
                x24 = (
                    x2h[:, :, tt, :]
                    .unsqueeze(2)
                    .broadcast_to([TT, K, OH, C])
                )
                e4 = eex[:, sp, :, 0, :].rearrange("p k (o i) -> p k o i", o=OH)
                ex4 = eex[:, sp, :, 1, :].rearrange("p k (o i) -> p k o i", o=OH)
                nc.vector.tensor_mul(ex4, e4, x24)

                if sp == 1:
                    # k-sum trees for den (over e) and num (over EX) for the
                    # whole pair; (sn, q) ride together in each wide op.
                    ev = eex[:].rearrange("p a k s q -> p a k (s q)")
                    t1 = tpool.tile([TT, 2, 3, 2 * SLAB], bf16, tag="t1")
                    nc.vector.tensor_add(t1[:], ev[:, :, 0:6:2], ev[:, :, 1:6:2])
                    t2 = tpool.tile([TT, 2, 2 * SLAB], bf16, tag="t2")
                    nc.vector.tensor_add(t2[:], t1[:, :, 0], t1[:, :, 1])
                    t3 = tpool.tile([TT, 2, 2 * SLAB], bf16, tag="t3")
                    nc.vector.tensor_add(t3[:], t1[:, :, 2], ev[:, :, 6])
                    nd = spool.tile([TT, 2, 2, SLAB], bf16, tag="nd")
                    nc.vector.tensor_add(
                        nd[:],
                        t2[:].rearrange("p a (s q) -> p a s q", s=2),
                        t3[:].rearrange("p a (s q) -> p a s q", s=2),
                    )
                    den = nd[:, :, 0]
                    num = nd[:, :, 1]

                    # r = 1/den in bf16: exponent-flip seed (int16 bit trick,
                    # 4x tensor_scalar) + one Newton step r1 = r0*(2 - d*r0).
                    r0 = spool.tile([TT, 2, SLAB], bf16, tag="r0")
                    nc.vector.tensor_scalar(
                        r0[:].bitcast(i16), den.bitcast(i16),
                        -1, RECIP_MAGIC, op0=mult, op1=add,
                    )
                    m = spool.tile([TT, 2, SLAB], bf16, tag="m")
                    nc.vector.tensor_mul(m[:], den, r0[:])
                    nc.vector.tensor_scalar(m[:], m[:], -1.0, 2.0, op0=mult, op1=add)
                    r1 = spool.tile([TT, 2, SLAB], bf16, tag="r1")
                    nc.vector.tensor_mul(r1[:], m[:], r0[:])

                    # y[t,o] = sum_i num * r (one 2x i-halving, then reduce)
                    y1 = spool.tile([TT, 2, SLAB], bf16, tag="y1")
                    nc.vector.tensor_mul(y1[:], num, r1[:])
                    y4 = y1[:].rearrange("p u (o h i) -> p u o h i", o=OH, h=2)
                    yh = spool.tile([TT, 2, OH, C // 2], bf16, tag="yh")
                    nc.vector.tensor_add(yh[:], y4[:, :, :, 0], y4[:, :, :, 1])
                    nc.vector.tensor_reduce(
                        y_sb[:, (tt - 1) * OH : (tt + 1) * OH],
                        yh[:],
                        axis=mybir.AxisListType.X,
                        op=mybir.AluOpType.add,
                    )

                if (tt + 1) % 8 == 0 or tt == nt - 1:
                    g0 = (tt // 8) * 8 * OH
                    nc.gpsimd.dma_start(
                        y_d[:, g0 : (tt + 1) * OH], y_sb[:, g0 : (tt + 1) * OH]
                    )

    nc.compile()
    return nc


def _prep_inputs(x, W, b):
    """Host-side scatter: per-core input dicts (pure layout/slicing)."""
    import ml_dtypes

    bf = ml_dtypes.bfloat16
    scale = np.float32(1.0 / np.sqrt(K))
    halves = []
    for h in range(2):
        Wh = W[h * OH * C * K : (h + 1) * OH * C * K]  # [OH*C*K, C, K]
        # rows (j,c) -> j*32+c ; cols (k,o,i) -> k*512 + o*32 + i
        Wp = (
            Wh.reshape(OH, C, K, C, K).transpose(4, 3, 2, 0, 1).reshape(K * C, FREE)
            * scale
        )
        bh = (
            b[h * OH * C * K : (h + 1) * OH * C * K]
            .reshape(OH, C, K)
            .transpose(2, 0, 1)
            .reshape(FREE)
            * scale
        )
        w1 = np.ascontiguousarray(Wp[:CD1])
        w2 = np.ascontiguousarray(
            np.concatenate([Wp[CD1:], bh[None, :]], axis=0)
        )
        halves.append((w1.astype(bf), w2.astype(bf)))

    t_len = x.shape[-1]
    nt = t_len // TT
    xs = []
    for bi in range(B):
        xp = np.zeros((C + 1, t_len + 8), dtype=np.float32)
        xp[:C, PAD : PAD + t_len] = x[bi]
        xp[C, PAD : PAD + t_len] = 1.0
        xt = np.ascontiguousarray(
            x[bi].T.reshape(nt, TT, C).transpose(1, 0, 2).reshape(TT, nt * C)
        )
        xs.append((xp.astype(bf), xt.astype(bf)))

    in_maps = []
    for core in range(8):
        bi, h = divmod(core, 2)
        w1, w2 = halves[h]
        xp, xt = xs[bi]
        in_maps.append({"xpad": xp, "xt": xt, "wp1": w1, "wp2": w2})
    return in_maps


def _assemble(results, t_len):
    """Gather per-core [TT, nt*OH] outputs into [B, O_FULL, t_len]."""
    nt = t_len // TT
    y = np.empty((B, O_FULL, t_len), dtype=np.float32)
    for core, res in enumerate(results):
        bi, h = divmod(core, 2)
        arr = res["yout"].reshape(TT, nt, OH)  # [tp, tt, o]
        y[bi, h * OH : (h + 1) * OH, :] = arr.transpose(2, 1, 0).reshape(OH, t_len)
    return y


def _run(x, W, b, trace=False, trace_cores=None):
    from concourse.bass_utils import run_bass_kernel_spmd
    from concourse.bass_interp import get_hw_module

    t_len = x.shape[-1]
    key = ("prog", t_len)
    if key not in _prog_cache:
        nc = _build(t_len)
        nc.m = get_hw_module(nc.m)
        _prog_cache[key] = nc
    nc = _prog_cache[key]

    in_maps = _prep_inputs(x, W, b)
    res = run_bass_kernel_spmd(
        nc,
        in_maps,
        core_ids=list(range(8)),
        trace=trace,
        trace_cores=trace_cores,
    )
    return _assemble(res.results, t_len), res


def kernel(x, W, b):
    y, _ = _run(np.asarray(x), np.asarray(W), np.asarray(b))
    return y


# revision 21
# speedup vs baseline: 1.0280x; 1.0280x over previous
"""DynamicConv1D Trainium2 kernel.

Reference computation (per batch b):
  dw = conv1d(x, W, pad=3) + b            # [O*I*K, T] dynamic weights
  dw = softmax(dw.reshape(O,I,K,T)/sqrt(K), axis=K)
  y[o,t] = sum_{i,k} x[i, t+k-3] * dw[o,i,k,t]

Sharding: 8 cores = 4 batches x 2 halves of O (16 out-channels each).
Each core gets x[b] plus its half of the (rearranged) conv weights and
computes y[b, half*16:(half+1)*16, :]. No collectives; the host scatters
inputs and concatenates outputs.

Per-core layout (t-tile = 128 positions on partitions):
  conv as matmul: dw[t, (k,o,i)] = sum_{(j,c)} X1[(j,c), t] * W'[(j,c), (k,o,i)]
    X1[(j,c), u] = x[c, u+j-3]  (im2col built host-side, bf16); ones row in
    X1b so the bias rides as an extra W' row; 1/sqrt(K) folded into W'/b.
  psum is organized as 4 bank groups (2+2+2+1 banks) so ScalarE drains dw
  with four wide exp's per tile.  x_unf ships from the host as a
  [tp, tile, (k,i)] tensor (no on-device DMA transposes).
  eex is PAIR-sized [t, pair, k, {e,ex}, (o,i)] so every k-sum tree level
  runs as one wide tensor_tensor add per tile-pair (DVE 2x mode: all
  operands bf16, packed; scalar_tensor_tensor / f32 operands would drop
  to 1x on hardware).
  Tail per pair stays bf16: r = 1/den via exponent-flip seed (int16
  tensor_scalar, 4x) + one Newton step (2x muls), y1 = num*r (2x),
  y[t,o] = reduce_i y1 after one 2x i-halving.
"""

import numpy as np

B = 4
C = 32
K = 7
T = 4096
O_FULL = 32
OH = 16  # out-channels per core
PAD = 3
TT = 128  # t positions per tile (partition dim)
FREE = K * OH * C  # 3584, matmul free index = k*512 + o*32 + i
SLAB = OH * C  # 512, one k-slab
CD1 = 128  # (j, c) rows for j=0..3
CD2 = 97  # (j, c) rows for j=4..6 plus ones row
CHUNK = 512  # psum chunk (1 bank); FREE = 7*CHUNK
KI = K * C  # 224
RECIP_MAGIC = 0x7EF4  # bf16 exponent-flip reciprocal seed (top16 of 0x7EF477D5)

_prog_cache = {}


def _build(t_len):
    """Build and compile the per-core Bass program for sequence length t_len."""
    import concourse.tile as tile
    from concourse import bacc, mybir

    nt = t_len // TT
    nc = bacc.Bacc("TRN2", target_bir_lowering=False, debug=False, num_devices=1)
    f32 = mybir.dt.float32
    bf16 = mybir.dt.bfloat16
    i16 = mybir.dt.int16
    mult = mybir.AluOpType.mult
    add = mybir.AluOpType.add

    padw = t_len + 8
    xp_d = nc.dram_tensor("xpad", [C + 1, padw], bf16, kind="ExternalInput").ap()
    xt_d = nc.dram_tensor("xt", [TT, nt * C], bf16, kind="ExternalInput").ap()
    w1_d = nc.dram_tensor("wp1", [CD1, FREE], bf16, kind="ExternalInput").ap()
    w2_d = nc.dram_tensor("wp2", [CD2, FREE], bf16, kind="ExternalInput").ap()
    y_d = nc.dram_tensor("yout", [TT, nt * OH], f32, kind="ExternalOutput").ap()

    with tile.TileContext(nc) as tc:
        with (
            tc.tile_pool(name="const", bufs=1) as cpool,
            tc.tile_pool(name="ep", bufs=2) as epool,
            tc.tile_pool(name="tree", bufs=2) as tpool,
            tc.tile_pool(name="small", bufs=2) as spool,
            tc.tile_pool(name="psum", bufs=1, space="PSUM") as ppool,
        ):
            xp = cpool.tile([C + 1, padw], bf16, tag="xp")
            xt = cpool.tile([TT, nt, C], bf16, tag="xt")
            x1a = cpool.tile([CD1, t_len], bf16, tag="x1a")
            x1b = cpool.tile([CD2, t_len], bf16, tag="x1b")
            w1 = cpool.tile([CD1, FREE], bf16, tag="w1")
            w2 = cpool.tile([CD2, FREE], bf16, tag="w2")
            x2h = cpool.tile([TT, K, nt, C], bf16, tag="x2h")
            y_sb = cpool.tile([TT, nt * OH], f32, tag="ysb")

            # Startup: only xp/xt/W come from (8-core contended) HBM; the
            # im2col tensors x1a/x1b and the unfold x2h are fanned out from
            # xp/xt with on-chip SBUF->SBUF DMAs, head (first tiles) first.
            # DMA triggers stay off the scalar queue (they steal act time).
            hf = 1792
            nt_ = nt
            HB = min(8, nt_)  # tile blocks in the head fanout
            HC = min(1024, t_len)  # head columns for x1 strips
            nc.sync.dma_start(xp[:], xp_d)
            nc.gpsimd.dma_start(xt[:], xt_d)

            # x1 head strips: x1a rows (j,c) = xp[c, t+j]
            for j in range(K):
                tgt, r0 = (x1a, j * C) if j < 4 else (x1b, (j - 4) * C)
                nc.sync.dma_start(
                    tgt[r0 : r0 + C, 0:HC], xp[0:C, j : j + HC]
                )
            nc.sync.dma_start(x1b[CD2 - 1 : CD2, 0:HC], xp[C : C + 1, 3 : 3 + HC])
            # x2h head (tile blocks 0:HB): x2h[tp, tt, k*C:(k+1)*C] from xt
            # with a (k-3) partition shift; block-crossing rows come from the
            # neighbor block; out-of-range rows are zero padding.
            for k in range(K):
                dlt = k - PAD
                if dlt <= 0:
                    if dlt < 0:
                        nc.vector.memset(x2h[:, k, 0, :], 0.0)
                    nc.gpsimd.dma_start(
                        x2h[-dlt : TT, k, 0:HB, :], xt[0 : TT + dlt, 0:HB, :]
                    )
                    if dlt < 0:
                        nc.gpsimd.dma_start(
                            x2h[0:-dlt, k, 1:HB, :], xt[TT + dlt : TT, 0 : HB - 1, :]
                        )
                else:
                    nc.gpsimd.dma_start(
                        x2h[0 : TT - dlt, k, 0:HB, :], xt[dlt:TT, 0:HB, :]
                    )
                    nc.gpsimd.dma_start(
                        x2h[TT - dlt : TT, k, 0:HB, :], xt[0:dlt, 1 : HB + 1, :]
                    )
            # weights (tile 0's matmuls gate on these)
            nc.sync.dma_start(w1[:, 0:hf], w1_d[:, 0:hf])
            nc.gpsimd.dma_start(w2[:, 0:hf], w2_d[:, 0:hf])
            nc.sync.dma_start(w1[:, hf:], w1_d[:, hf:])
            nc.gpsimd.dma_start(w2[:, hf:], w2_d[:, hf:])

            # tails
            if HC < t_len:
                for j in range(K):
                    tgt, r0 = (x1a, j * C) if j < 4 else (x1b, (j - 4) * C)
                    nc.sync.dma_start(
                        tgt[r0 : r0 + C, HC:t_len], xp[0:C, j + HC : j + t_len]
                    )
                nc.sync.dma_start(
                    x1b[CD2 - 1 : CD2, HC:t_len], xp[C : C + 1, 3 + HC : 3 + t_len]
                )
            for k in range(K if HB < nt else 0):
                dlt = k - PAD
                if dlt <= 0:
                    nc.gpsimd.dma_start(
                        x2h[-dlt : TT, k, HB:nt, :], xt[0 : TT + dlt, HB:nt, :]
                    )
                    if dlt < 0:
                        nc.gpsimd.dma_start(
                            x2h[0:-dlt, k, HB:nt, :],
                            xt[TT + dlt : TT, HB - 1 : nt - 1, :],
                        )
                else:
                    nc.vector.memset(x2h[:, k, nt - 1, :], 0.0)
                    nc.gpsimd.dma_start(
                        x2h[0 : TT - dlt, k, HB:nt, :], xt[dlt:TT, HB:nt, :]
                    )
                    if HB + 1 <= nt - 1:
                        nc.gpsimd.dma_start(
                            x2h[TT - dlt : TT, k, HB : nt - 1, :],
                            xt[0:dlt, HB + 1 : nt, :],
                        )

            for tt in range(nt):
                t0 = tt * TT
                sp = tt % 2  # pair slot
                x1at = x1a[:, t0 : t0 + TT]
                x1bt = x1b[:, t0 : t0 + TT]

                # psum: 3 double-bank groups + 1 single bank (7 chunks of 512)
                pg = [
                    ppool.tile([TT, 1024], f32, tag="pA", name="pA"),
                    ppool.tile([TT, 1024], f32, tag="pB", name="pB"),
                    ppool.tile([TT, 1024], f32, tag="pC", name="pC"),
                    ppool.tile([TT, 512], f32, tag="pD", name="pD"),
                ]

                def chunk_ap(ci):
                    g, o = divmod(ci, 2)
                    return pg[g][:, o * 512 : (o + 1) * 512]

                # Interleave the two contraction halves per chunk so each
                # chunk (and its act) completes as early as possible.
                for ci in range(K):
                    cs = slice(ci * CHUNK, (ci + 1) * CHUNK)
                    nc.tensor.matmul(
                        chunk_ap(ci), x1at, w1[:, cs], start=True, stop=False
                    )
                    nc.tensor.matmul(
                        chunk_ap(ci), x1bt, w2[:, cs], start=False, stop=True
                    )

                # eex[t, pair, k, {e, ex}, (o,i)]
                if sp == 0:
                    eex = epool.tile([TT, 2, K, 2, SLAB], bf16, tag="eex")
                for g in range(4):
                    kw = 2 if g < 3 else 1  # k-slabs in this group
                    src = pg[g][:].rearrange("p (k q) -> p k q", k=kw)
                    nc.scalar.activation(
                        eex[:, sp, 2 * g : 2 * g + kw, 0, :],
                        src,
                        mybir.ActivationFunctionType.Exp,
                    )

                # EX = e * x_unf broadcast over o (one wide 2x TT op per tile)
                x24 = (
                    x2h[:, :, tt, :]
                    .unsqueeze(2)
                    .broadcast_to([TT, K, OH, C])
                )
                e4 = eex[:, sp, :, 0, :].rearrange("p k (o i) -> p k o i", o=OH)
                ex4 = eex[:, sp, :, 1, :].rearrange("p k (o i) -> p k o i", o=OH)
                nc.vector.tensor_mul(ex4, e4, x24)

                if sp == 1:
                    # k-sum trees for den (over e) and num (over EX) for the
                    # whole pair; (sn, q) ride together in each wide op.
                    ev = eex[:].rearrange("p a k s q -> p a k (s q)")
                    t1 = tpool.tile([TT, 2, 3, 2 * SLAB], bf16, tag="t1")
                    nc.vector.tensor_add(t1[:], ev[:, :, 0:6:2], ev[:, :, 1:6:2])
                    t2 = tpool.tile([TT, 2, 2 * SLAB], bf16, tag="t2")
                    nc.vector.tensor_add(t2[:], t1[:, :, 0], t1[:, :, 1])
                    t3 = tpool.tile([TT, 2, 2 * SLAB], bf16, tag="t3")
                    nc.vector.tensor_add(t3[:], t1[:, :, 2], ev[:, :, 6])
                    nd = spool.tile([TT, 2, 2, SLAB], bf16, tag="nd")
                    nc.vector.tensor_add(
                        nd[:],
                        t2[:].rearrange("p a (s q) -> p a s q", s=2),
                        t3[:].rearrange("p a (s q) -> p a s q", s=2),
                    )
                    den = nd[:, :, 0]
                    num = nd[:, :, 1]

                    # r = 1/den in bf16: exponent-flip seed (int16 bit trick,
                    # 4x tensor_scalar) + one Newton step r1 = r0*(2 - d*r0).
                    r0 = spool.tile([TT, 2, SLAB], bf16, tag="r0")
                    nc.vector.tensor_scalar(
                        r0[:].bitcast(i16), den.bitcast(i16),
                        -1, RECIP_MAGIC, op0=mult, op1=add,
                    )
                    m = spool.tile([TT, 2, SLAB], bf16, tag="m")
                    nc.vector.tensor_mul(m[:], den, r0[:])
                    nc.vector.tensor_scalar(m[:], m[:], -1.0, 2.0, op0=mult, op1=add)
                    r1 = spool.tile([TT, 2, SLAB], bf16, tag="r1")
                    nc.vector.tensor_mul(r1[:], m[:], r0[:])

                    # y[t,o] = sum_i num * r (one 2x i-halving, then reduce)
                    y1 = spool.tile([TT, 2, SLAB], bf16, tag="y1")
                    nc.vector.tensor_mul(y1[:], num, r1[:])
                    y4 = y1[:].rearrange("p u (o h i) -> p u o h i", o=OH, h=2)
                    yh = spool.tile([TT, 2, OH, C // 2], bf16, tag="yh")
                    nc.vector.tensor_add(yh[:], y4[:, :, :, 0], y4[:, :, :, 1])
                    nc.vector.tensor_reduce(
                        y_sb[:, (tt - 1) * OH : (tt + 1) * OH],
                        yh[:],
                        axis=mybir.AxisListType.X,
                        op=mybir.AluOpType.add,
                    )

                if (tt + 1) % 8 == 0 or tt == nt - 1:
                    g0 = (tt // 8) * 8 * OH
                    nc.gpsimd.dma_start(
                        y_d[:, g0 : (tt + 1) * OH], y_sb[:, g0 : (tt + 1) * OH]
                    )

    nc.compile()
    return nc


def _prep_inputs(x, W, b):
    """Host-side scatter: per-core input dicts (pure layout/slicing)."""
    import ml_dtypes

    bf = ml_dtypes.bfloat16
    scale = np.float32(1.0 / np.sqrt(K))
    halves = []
    for h in range(2):
        Wh = W[h * OH * C * K : (h + 1) * OH * C * K]  # [OH*C*K, C, K]
        # rows (j,c) -> j*32+c ; cols (k,o,i) -> k*512 + o*32 + i
        Wp = (
            Wh.reshape(OH, C, K, C, K).transpose(4, 3, 2, 0, 1).reshape(K * C, FREE)
            * scale
        )
        bh = (
            b[h * OH * C * K : (h + 1) * OH * C * K]
            .reshape(OH, C, K)
            .transpose(2, 0, 1)
            .reshape(FREE)
            * scale
        )
        w1 = np.ascontiguousarray(Wp[:CD1])
        w2 = np.ascontiguousarray(
            np.concatenate([Wp[CD1:], bh[None, :]], axis=0)
        )
        halves.append((w1.astype(bf), w2.astype(bf)))

    t_len = x.shape[-1]
    nt = t_len // TT
    xs = []
    for bi in range(B):
        xp = np.zeros((C + 1, t_len + 8), dtype=np.float32)
        xp[:C, PAD : PAD + t_len] = x[bi]
        xp[C, PAD : PAD + t_len] = 1.0
        xt = np.ascontiguousarray(
            x[bi].T.reshape(nt, TT, C).transpose(1, 0, 2).reshape(TT, nt * C)
        )
        xs.append((xp.astype(bf), xt.astype(bf)))

    in_maps = []
    for core in range(8):
        bi, h = divmod(core, 2)
        w1, w2 = halves[h]
        xp, xt = xs[bi]
        in_maps.append({"xpad": xp, "xt": xt, "wp1": w1, "wp2": w2})
    return in_maps


def _assemble(results, t_len):
    """Gather per-core [TT, nt*OH] outputs into [B, O_FULL, t_len]."""
    nt = t_len // TT
    y = np.empty((B, O_FULL, t_len), dtype=np.float32)
    for core, res in enumerate(results):
        bi, h = divmod(core, 2)
        arr = res["yout"].reshape(TT, nt, OH)  # [tp, tt, o]
        y[bi, h * OH : (h + 1) * OH, :] = arr.transpose(2, 1, 0).reshape(OH, t_len)
    return y


def _run(x, W, b, trace=False, trace_cores=None):
    from concourse.bass_utils import run_bass_kernel_spmd
    from concourse.bass_interp import get_hw_module

    t_len = x.shape[-1]
    key = ("prog", t_len)
    if key not in _prog_cache:
        nc = _build(t_len)
        nc.m = get_hw_module(nc.m)
        _prog_cache[key] = nc
    nc = _prog_cache[key]

    in_maps = _prep_inputs(x, W, b)
    res = run_bass_kernel_spmd(
        nc,
        in_maps,
        core_ids=list(range(8)),
        trace=trace,
        trace_cores=trace_cores,
    )
    return _assemble(res.results, t_len), res


def kernel(x, W, b):
    y, _ = _run(np.asarray(x), np.asarray(W), np.asarray(b))
    return y
